# revision 13
# baseline (speedup 1.0000x reference)
"""Trainium2 Bass kernel for the AnalyticalBatteryRNNCell batch step.

Takes FULL inputs (B=2097152 samples), shards the batch across 8 NeuronCores
(pure data parallel), runs a Tile-scheduled elementwise kernel per core, and
gathers the full output: (V[B,1], X_new[B,8]).

Math restructuring vs the jax reference (exact algebra, f32 rounding only):
  - rsqrt(u)        = exp(-0.5*ln(u))          [single ACT table set ln/exp]
  - asinh(z)        = ln(z + exp(0.5*ln(z^2+1)))
  - Redlich-Kister sum for the positive electrode collapses to a single
    degree-13 polynomial R(t) = sum_{j>=1} e_j t^j (constant folded into U0P),
    with e_j = ((j+1)/2)*(Ap[j-1]-Ap[j+1]); evaluated by a fused
    scalar_tensor_tensor Horner chain: acc <- (acc + b_k) * t.
  - negative-electrode RK sum (K=1) is affine in qnS_new and folded into the
    final linear combination.

Schedule: two software-pipelined stages per tile (front / tail), emitted with
a one-tile skew so each engine's in-order queue always has ready work.
Engines: ACT = transcendentals + affine scale/bias; DVE = fused
scalar_tensor_tensor chains (HW-only there); Pool = plain tensor_tensor.
"""
import sys
import numpy as np

for _p in ("/opt/trn_rl_repo", "/root/.axon_site/_ro/trn_rl_repo"):
    if _p not in sys.path:
        sys.path.insert(0, _p)

import concourse.bass as bass
import concourse.bacc as bacc
import concourse.mybir as mybir
import concourse.tile as tile
from concourse import bass_utils

F32 = mybir.dt.float32
AF = mybir.ActivationFunctionType
OP = mybir.AluOpType

# All ACT functions used here (Square/Ln/Exp/Identity/Copy) live in the
# `natural_log_exp_and_others` table set. The stock insert_act_table_loads
# pass picks the FIRST set containing each function, which alternates
# exp_and_others <-> natural_log and reloads the table (~2.7us) on every
# Ln/Exp transition. Restrict the pass to the one covering set (index order
# preserved so act_func_set_id still matches act_info.json).
_ACT_KEEP_SET = "natural_log_exp_and_others"


def _patched_insert_act_table_loads(self):
    import bass_rust as _br
    from concourse.hw_specs import get_activation_tables
    has_activation = any(
        isinstance(i, mybir.InstActivation)
        for b in self.main_func.blocks
        for i in b.instructions
    )
    if not has_activation:
        return
    tables = [
        (nm, (s if nm == _ACT_KEEP_SET else set()))
        for nm, s in get_activation_tables(self.m.arch).items()
    ]
    _br.insert_act_table_loads(self, tables)


bacc.Bacc.insert_act_table_loads = _patched_insert_act_table_loads

B_FULL = 2_097_152
N_CORES = 8
SH = B_FULL // N_CORES          # 262144 samples per core
P = 128
N = 512                         # free elems per partition per tile
T = SH // (P * N)
assert T * P * N == SH

QMOBILE = 7600.0
XN_MAX, XN_MIN = 0.6, 0.0
RO = 0.117215
R_GAS = 8.3144621
F_C = 96487.0
ALPHA = 0.5
SN, SP = 0.000437545, 0.00030962
KN, KP = 2120.96, 248898.0
VOL = 2.0e-5
VOL_S_FRAC = 0.1
T_DIFF = 7.0e6
TO, TSN, TSP = 6.08671, 1001.38, 46.4311
U0P, U0N = 4.03, 0.01
BASE_AP = np.array([-31593.7, 0.106747, 24606.4, -78561.9, 13317.9, 307387.0,
                    84916.1, -1.07469e6, 2285.04, 990894.0, 283920.0,
                    -161513.0, -469218.0], dtype=np.float64)
BASE_AN0 = 86.19


def _derived_consts(Ap_mult, An0_mult):
    qMax = QMOBILE / (XN_MAX - XN_MIN)
    VolS = VOL_S_FRAC * VOL
    VolB = VOL - VolS
    qSMax = qMax * VolS / VOL
    c = {}
    c["c1"] = 1.0 / qSMax
    cT = R_GAS / (F_C * ALPHA)
    c["cT2"] = R_GAS / F_C
    c["c2n"] = 1.0 / (SN * 2.0 * KN)
    c["c2p"] = 1.0 / (SP * 2.0 * KP)
    c["c4"] = 1.0 / (VolB * T_DIFF)
    c["c5"] = 1.0 / (VolS * T_DIFF)
    c["aVo"], c["bVo"] = 1.0 - 1.0 / TO, RO / TO
    c["aVsn"], c["bVsn_cT"] = 1.0 - 1.0 / TSN, cT / TSN
    c["aVsp"], c["bVsp_cT"] = 1.0 - 1.0 / TSP, cT / TSP

    Ap = np.float64(np.asarray(Ap_mult, dtype=np.float64)) * BASE_AP
    An0 = float(np.asarray(An0_mult).reshape(-1)[0]) * BASE_AN0
    A = np.zeros(15)
    A[:13] = Ap
    e = np.zeros(14)
    for j in range(14):
        am1 = A[j - 1] if j >= 1 else 0.0
        e[j] = 0.5 * (j + 1) * (am1 - A[j + 1])
    c["etil"] = e / F_C
    c["cA"] = 2.0 * An0 * c["c1"] / F_C
    c["Kc"] = (U0P - U0N) + e[0] / F_C + An0 / F_C
    return c


def build_nc(c, reps=1):
    nc = bacc.Bacc(
        "TRN2", target_bir_lowering=False, debug=False, num_devices=N_CORES
    )

    f = lambda x: float(x)
    c1, c4, c5 = f(c["c1"]), f(c["c4"]), f(c["c5"])
    etil = c["etil"]

    act_biases = [float(np.log(c["c2n"])), float(np.log(c["c2p"])), f(c["Kc"])]
    for val in act_biases:
        if val in (0.0, 1.0):
            continue
        tns = nc.alloc_sbuf_tensor(f"constb-{val!r}", [128, 1], F32)
        nc.gpsimd.memset(tns.ap(), val)
        nc.const_aps.aps[(F32, val)] = tns.ap()
    nc.all_engine_barrier()

    inp_d = nc.dram_tensor("inp", [SH, 1], F32, kind="ExternalInput")
    st_d = nc.dram_tensor("st", [SH, 8], F32, kind="ExternalInput")
    v_d = nc.dram_tensor("V", [SH, 1], F32, kind="ExternalOutput")
    x_d = nc.dram_tensor("X", [SH, 8], F32, kind="ExternalOutput")

    st_v = st_d[:].rearrange("(t p n) c -> t p (n c)", p=P, n=N)
    x_v = x_d[:].rearrange("(t p n) c -> t p (n c)", p=P, n=N)
    inp_v = inp_d[:].rearrange("(t p n) c -> t p (n c)", p=P, n=N)
    v_v = v_d[:].rearrange("(t p n) c -> t p (n c)", p=P, n=N)

    def act_affine(out, in_, s0, s1=0.0):
        if s1 == 0.0:
            nc.scalar.activation(out, in_, AF.Copy, bias=0.0, scale=f(s0))
        else:
            nc.scalar.activation(out, in_, AF.Identity, bias=f(s1), scale=f(s0))

    with tile.TileContext(nc) as tc:
        with (
            tc.tile_pool(name="sin", bufs=3) as sin,
            tc.tile_pool(name="xo", bufs=3) as xo,
            tc.tile_pool(name="pairs", bufs=2) as pp,
            tc.tile_pool(name="mids", bufs=3) as mp,
            tc.tile_pool(name="units", bufs=2) as up,
        ):
            state = {}       # per-tile live tiles passed stage1 -> stage2

            def load(t):
                S = sin.tile([P, N * 8], F32, tag="S")
                I = sin.tile([P, N], F32, tag="I")
                nc.sync.dma_start(S[:], st_v[t])
                nc.sync.dma_start(I[:], inp_v[t])
                state[t] = {"S": S, "I": I}

            def stage1(t):
                st_ = state[t]
                S, I = st_["S"], st_["I"]
                X = xo.tile([P, N * 8], F32, tag="X")
                st_["X"] = X
                S3 = S[:].rearrange("p (n c) -> p n c", c=8)
                S4 = S[:].rearrange("p (n a b) -> p n a b", a=4, b=2)
                X3 = X[:].rearrange("p (n c) -> p n c", c=8)
                X4 = X[:].rearrange("p (n a b) -> p n a b", a=4, b=2)
                S_qS, S_qB = S4[:, :, 2:4, 1], S4[:, :, 2:4, 0]
                X_qS, X_qB = X4[:, :, 2:4, 1], X4[:, :, 2:4, 0]
                st_["X3"] = X3

                # transcendental front
                tA = pp.tile([P, N, 2], F32, tag="tA")
                nc.scalar.activation(tA[:], S_qS, AF.Square, scale=c1)
                nc.vector.scalar_tensor_tensor(tA[:], S_qS, c1, tA[:],
                                               OP.mult, OP.subtract)
                nc.scalar.activation(tA[:], tA[:], AF.Ln)
                nc.scalar.activation(tA[:, :, 0], tA[:, :, 0], AF.Exp,
                                     scale=-0.5, bias=float(np.log(c["c2n"])))
                nc.scalar.activation(tA[:, :, 1], tA[:, :, 1], AF.Exp,
                                     scale=-0.5, bias=float(np.log(c["c2p"])))
                tZ = pp.tile([P, N, 2], F32, tag="tZ")
                nc.gpsimd.tensor_tensor(tZ[:, :, 0], I[:], tA[:, :, 0], OP.mult)
                nc.gpsimd.tensor_tensor(tZ[:, :, 1], I[:], tA[:, :, 1], OP.mult)
                tC = pp.tile([P, N, 2], F32, tag="tC")
                nc.scalar.activation(tC[:], tZ[:], AF.Square)
                nc.scalar.activation(tC[:], tC[:], AF.Ln, bias=1.0)
                nc.scalar.activation(tC[:], tC[:], AF.Exp, scale=0.5)
                nc.gpsimd.tensor_tensor(tZ[:], tZ[:], tC[:], OP.add)
                nc.scalar.activation(tZ[:], tZ[:], AF.Ln)

                # overpotentials
                tW = pp.tile([P, N, 2], F32, tag="tW")
                act_affine(tW[:, :, 0], S3[:, :, 0], c["bVsn_cT"])
                act_affine(tW[:, :, 1], S3[:, :, 0], c["bVsp_cT"])
                nc.gpsimd.tensor_tensor(tZ[:], tW[:], tZ[:], OP.mult)
                nc.vector.scalar_tensor_tensor(X3[:, :, 2], S3[:, :, 2], f(c["aVsn"]),
                                               tZ[:, :, 0], OP.mult, OP.add)
                nc.vector.scalar_tensor_tensor(X3[:, :, 3], S3[:, :, 3], f(c["aVsp"]),
                                               tZ[:, :, 1], OP.mult, OP.add)
                tVo = up.tile([P, N], F32, tag="tVo")
                act_affine(tVo[:], S3[:, :, 1], c["aVo"])
                nc.vector.scalar_tensor_tensor(X3[:, :, 1], I[:], f(c["bVo"]),
                                               tVo[:], OP.mult, OP.add)
                nc.vector.tensor_copy(X3[:, :, 0], S3[:, :, 0])   # Tb

                # charge updates
                tG = pp.tile([P, N, 2], F32, tag="tG")
                tH = pp.tile([P, N, 2], F32, tag="tH")
                act_affine(tG[:], S_qB, 1.0 - c4)
                nc.vector.scalar_tensor_tensor(X_qB, S_qS, c5, tG[:], OP.mult, OP.add)
                act_affine(tH[:], S_qS, 1.0 - c5)
                nc.vector.scalar_tensor_tensor(tH[:], S_qB, c4, tH[:], OP.mult, OP.add)
                nc.gpsimd.tensor_tensor(X3[:, :, 5], tH[:, :, 0], I[:], OP.subtract)
                nc.gpsimd.tensor_tensor(X3[:, :, 7], tH[:, :, 1], I[:], OP.add)

                # wh = cT2*Tb (read S now; consumed in stage2)
                wh = mp.tile([P, N], F32, tag="wh")
                act_affine(wh[:], S3[:, :, 0], c["cT2"])
                st_["wh"] = wh

            def stage2(t):
                st_ = state.pop(t)
                X3, wh = st_["X3"], st_["wh"]
                X = st_["X"]
                Vt = xo.tile([P, N], F32, tag="Vt", name="Vt")
                X4 = X[:].rearrange("p (n a b) -> p n a b", a=4, b=2)
                X_qS = X4[:, :, 2:4, 1]

                # t_p and Horner chain (DVE)
                tp = up.tile([P, N], F32, tag="tp")
                nc.vector.tensor_scalar(tp[:], X3[:, :, 7], 2.0 * c1, 1.0,
                                        OP.mult, OP.subtract)
                acc = up.tile([P, N], F32, tag="acc")
                nc.vector.tensor_scalar_mul(acc[:], tp[:], f(etil[13]))
                for k in range(2, 14):
                    nc.vector.scalar_tensor_tensor(acc[:], acc[:], f(etil[14 - k]),
                                                   tp[:], OP.add, OP.mult)

                # logits
                tL1 = pp.tile([P, N, 2], F32, tag="tL1")
                tL2 = pp.tile([P, N, 2], F32, tag="tL2")
                nc.scalar.activation(tL1[:], X_qS, AF.Ln, scale=c1)
                nc.scalar.activation(tL2[:], X_qS, AF.Ln, scale=-c1, bias=1.0)
                nc.vector.tensor_tensor(tL2[:], tL2[:], tL1[:], OP.subtract)
                dd = up.tile([P, N], F32, tag="dd")
                nc.gpsimd.tensor_tensor(dd[:], tL2[:, :, 1], tL2[:, :, 0], OP.subtract)
                nc.gpsimd.tensor_tensor(dd[:], wh[:], dd[:], OP.mult)

                sm = up.tile([P, N], F32, tag="sm")
                nc.gpsimd.tensor_tensor(sm[:], X3[:, :, 1], X3[:, :, 2], OP.add)
                nc.gpsimd.tensor_tensor(sm[:], sm[:], X3[:, :, 3], OP.add)
                act_affine(sm[:], sm[:], -1.0, c["Kc"])
                nc.vector.scalar_tensor_tensor(acc[:], X3[:, :, 5], f(-c["cA"]),
                                               acc[:], OP.mult, OP.add)
                nc.gpsimd.tensor_tensor(acc[:], acc[:], dd[:], OP.add)
                nc.vector.tensor_tensor(Vt[:], acc[:], sm[:], OP.add)

                nc.sync.dma_start(x_v[t], X[:])
                nc.sync.dma_start(v_v[t], Vt[:])

            # software-pipelined emission: loads 1 tick ahead, stage2 skewed
            def whole():
                load(0)
                for k in range(T + 1):
                    if k + 1 < T:
                        load(k + 1)
                    if k >= 1:
                        stage2(k - 1)
                    if k < T:
                        stage1(k)

            if reps == 1:
                whole()
            else:
                with tc.For_i(0, reps, 1):
                    whole()
    nc.compile()
    return nc


def _run(inputs, states, Ap_mult, An0_mult, trace=False):
    c = _derived_consts(Ap_mult, An0_mult)
    nc = build_nc(c)
    inputs = np.ascontiguousarray(np.asarray(inputs, dtype=np.float32))
    states = np.ascontiguousarray(np.asarray(states, dtype=np.float32))
    in_maps = [
        {"inp": inputs[k * SH:(k + 1) * SH], "st": states[k * SH:(k + 1) * SH]}
        for k in range(N_CORES)
    ]
    res = bass_utils.run_bass_kernel_spmd(
        nc, in_maps, core_ids=list(range(N_CORES)), trace=trace
    )
    V = np.concatenate([res.results[k]["V"] for k in range(N_CORES)], axis=0)
    X = np.concatenate([res.results[k]["X"] for k in range(N_CORES)], axis=0)
    return (V, X), res


def kernel(inputs, states, Ap_mult, An0_mult):
    (V, X), _ = _run(inputs, states, Ap_mult, An0_mult, trace=False)
    return V, X


if __name__ == "__main__":
    rng = np.random.default_rng(0)
    inputs = (0.5 + 2.5 * rng.random((B_FULL, 1))).astype(np.float32)
    qMax = QMOBILE / 0.6
    qS, qB = qMax * 0.1, qMax * 0.9
    xn = (0.25 + 0.3 * rng.random(B_FULL)).astype(np.float32)
    xp = (0.45 + 0.3 * rng.random(B_FULL)).astype(np.float32)
    states = np.stack([
        292.1 + 0.5 * rng.random(B_FULL), 0.1 * rng.random(B_FULL),
        0.05 * rng.random(B_FULL), 0.05 * rng.random(B_FULL),
        xn * qB, xn * qS, xp * qB, xp * qS], axis=1).astype(np.float32)
    V, X = kernel(inputs, states, np.ones(13, np.float32), np.ones(1, np.float32))
    print("V", V.shape, V[:4, 0])
    print("X", X.shape, X[0])


# revision 17
# speedup vs baseline: 735.4003x; 735.4003x over previous
"""Trainium2 Bass kernel for the AnalyticalBatteryRNNCell batch step.

Takes FULL inputs (B=2097152 samples), shards the batch across 8 NeuronCores
(pure data parallel), runs a Tile-scheduled elementwise kernel per core, and
gathers the full output: (V[B,1], X_new[B,8]).

Math restructuring vs the jax reference (exact algebra, f32 rounding only):
  - rsqrt(u)        = exp(-0.5*ln(u))          [single ACT table set ln/exp]
  - asinh(z)        = ln(z + exp(0.5*ln(z^2+1)))
  - Redlich-Kister sum for the positive electrode collapses to a single
    degree-13 polynomial R(t) = sum_{j>=1} e_j t^j (constant folded into U0P),
    with e_j = ((j+1)/2)*(Ap[j-1]-Ap[j+1]); evaluated by a fused
    scalar_tensor_tensor Horner chain: acc <- (acc + b_k) * t.
  - negative-electrode RK sum (K=1) is affine in qnS_new and folded into the
    final linear combination.

Schedule: two software-pipelined stages per tile (front / tail), emitted with
a one-tile skew so each engine's in-order queue always has ready work.
Engines: ACT = transcendentals + affine scale/bias; DVE = fused
scalar_tensor_tensor chains (HW-only there); Pool = plain tensor_tensor.
"""
import sys
import numpy as np

for _p in ("/opt/trn_rl_repo", "/root/.axon_site/_ro/trn_rl_repo"):
    if _p not in sys.path:
        sys.path.insert(0, _p)

import concourse.bass as bass
import concourse.bacc as bacc
import concourse.mybir as mybir
import concourse.tile as tile
from concourse import bass_utils

F32 = mybir.dt.float32
AF = mybir.ActivationFunctionType
OP = mybir.AluOpType

# All ACT functions used here (Square/Ln/Exp/Identity/Copy) live in the
# `natural_log_exp_and_others` table set. The stock insert_act_table_loads
# pass picks the FIRST set containing each function, which alternates
# exp_and_others <-> natural_log and reloads the table (~2.7us) on every
# Ln/Exp transition. Restrict the pass to the one covering set (index order
# preserved so act_func_set_id still matches act_info.json).
_ACT_KEEP_SET = "natural_log_exp_and_others"


def _patched_insert_act_table_loads(self):
    import bass_rust as _br
    from concourse.hw_specs import get_activation_tables
    has_activation = any(
        isinstance(i, mybir.InstActivation)
        for b in self.main_func.blocks
        for i in b.instructions
    )
    if not has_activation:
        return
    tables = [
        (nm, (s if nm == _ACT_KEEP_SET else set()))
        for nm, s in get_activation_tables(self.m.arch).items()
    ]
    _br.insert_act_table_loads(self, tables)


bacc.Bacc.insert_act_table_loads = _patched_insert_act_table_loads

B_FULL = 2_097_152
N_CORES = 8
SH = B_FULL // N_CORES          # 262144 samples per core
P = 128
N = 512                         # free elems per partition per tile
T = SH // (P * N)
assert T * P * N == SH

QMOBILE = 7600.0
XN_MAX, XN_MIN = 0.6, 0.0
RO = 0.117215
R_GAS = 8.3144621
F_C = 96487.0
ALPHA = 0.5
SN, SP = 0.000437545, 0.00030962
KN, KP = 2120.96, 248898.0
VOL = 2.0e-5
VOL_S_FRAC = 0.1
T_DIFF = 7.0e6
TO, TSN, TSP = 6.08671, 1001.38, 46.4311
U0P, U0N = 4.03, 0.01
BASE_AP = np.array([-31593.7, 0.106747, 24606.4, -78561.9, 13317.9, 307387.0,
                    84916.1, -1.07469e6, 2285.04, 990894.0, 283920.0,
                    -161513.0, -469218.0], dtype=np.float64)
BASE_AN0 = 86.19


def _derived_consts(Ap_mult, An0_mult):
    qMax = QMOBILE / (XN_MAX - XN_MIN)
    VolS = VOL_S_FRAC * VOL
    VolB = VOL - VolS
    qSMax = qMax * VolS / VOL
    c = {}
    c["c1"] = 1.0 / qSMax
    cT = R_GAS / (F_C * ALPHA)
    c["cT2"] = R_GAS / F_C
    c["c2n"] = 1.0 / (SN * 2.0 * KN)
    c["c2p"] = 1.0 / (SP * 2.0 * KP)
    c["c4"] = 1.0 / (VolB * T_DIFF)
    c["c5"] = 1.0 / (VolS * T_DIFF)
    c["aVo"], c["bVo"] = 1.0 - 1.0 / TO, RO / TO
    c["aVsn"], c["bVsn_cT"] = 1.0 - 1.0 / TSN, cT / TSN
    c["aVsp"], c["bVsp_cT"] = 1.0 - 1.0 / TSP, cT / TSP

    Ap = np.float64(np.asarray(Ap_mult, dtype=np.float64)) * BASE_AP
    An0 = float(np.asarray(An0_mult).reshape(-1)[0]) * BASE_AN0
    A = np.zeros(15)
    A[:13] = Ap
    e = np.zeros(14)
    for j in range(14):
        am1 = A[j - 1] if j >= 1 else 0.0
        e[j] = 0.5 * (j + 1) * (am1 - A[j + 1])
    c["etil"] = e / F_C
    c["cA"] = 2.0 * An0 * c["c1"] / F_C
    c["Kc"] = (U0P - U0N) + e[0] / F_C + An0 / F_C
    return c


def build_nc(c, reps=1):
    nc = bacc.Bacc(
        "TRN2", target_bir_lowering=False, debug=False, num_devices=N_CORES
    )

    f = lambda x: float(x)
    c1, c4, c5 = f(c["c1"]), f(c["c4"]), f(c["c5"])
    etil = c["etil"]

    act_biases = [float(np.log(c["c2n"])), float(np.log(c["c2p"])), f(c["Kc"])]
    for val in act_biases:
        if val in (0.0, 1.0):
            continue
        tns = nc.alloc_sbuf_tensor(f"constb-{val!r}", [128, 1], F32)
        nc.gpsimd.memset(tns.ap(), val)
        nc.const_aps.aps[(F32, val)] = tns.ap()
    nc.all_engine_barrier()

    inp_d = nc.dram_tensor("inp", [SH, 1], F32, kind="ExternalInput")
    st_d = nc.dram_tensor("st", [SH, 8], F32, kind="ExternalInput")
    v_d = nc.dram_tensor("V", [SH, 1], F32, kind="ExternalOutput")
    x_d = nc.dram_tensor("X", [SH, 8], F32, kind="ExternalOutput")

    st_v = st_d[:].rearrange("(t p n) c -> t p (n c)", p=P, n=N)
    x_v = x_d[:].rearrange("(t p n) c -> t p (n c)", p=P, n=N)
    inp_v = inp_d[:].rearrange("(t p n) c -> t p (n c)", p=P, n=N)
    v_v = v_d[:].rearrange("(t p n) c -> t p (n c)", p=P, n=N)

    def act_affine(out, in_, s0, s1=0.0):
        if s1 == 0.0:
            nc.scalar.activation(out, in_, AF.Copy, bias=0.0, scale=f(s0))
        else:
            nc.scalar.activation(out, in_, AF.Identity, bias=f(s1), scale=f(s0))

    SKEW = 2     # stage2(t) emitted SKEW ticks after stage1(t)
    with tile.TileContext(nc) as tc:
        with (
            tc.tile_pool(name="sin", bufs=2) as sin,
            tc.tile_pool(name="xo", bufs=SKEW + 2) as xo,
            tc.tile_pool(name="pairs", bufs=2) as pp,
            tc.tile_pool(name="mids", bufs=SKEW + 2) as mp,
            tc.tile_pool(name="units", bufs=2) as up,
        ):
            state = {}       # per-tile live tiles passed stage1 -> stage2

            def load(t):
                S = sin.tile([P, N * 8], F32, tag="S")
                I = sin.tile([P, N], F32, tag="I")
                nc.sync.dma_start(S[:], st_v[t])
                nc.sync.dma_start(I[:], inp_v[t])
                state[t] = {"S": S, "I": I}

            def stage1(t):
                st_ = state[t]
                S, I = st_["S"], st_["I"]
                X = xo.tile([P, N * 8], F32, tag="X")
                st_["X"] = X
                S3 = S[:].rearrange("p (n c) -> p n c", c=8)
                S4 = S[:].rearrange("p (n a b) -> p n a b", a=4, b=2)
                X3 = X[:].rearrange("p (n c) -> p n c", c=8)
                X4 = X[:].rearrange("p (n a b) -> p n a b", a=4, b=2)
                S_qS, S_qB = S4[:, :, 2:4, 1], S4[:, :, 2:4, 0]
                X_qS, X_qB = X4[:, :, 2:4, 1], X4[:, :, 2:4, 0]
                st_["X3"] = X3

                # transcendental front
                tA = pp.tile([P, N, 2], F32, tag="tA")
                nc.scalar.activation(tA[:], S_qS, AF.Square, scale=c1)
                nc.vector.scalar_tensor_tensor(tA[:], S_qS, c1, tA[:],
                                               OP.mult, OP.subtract)
                nc.scalar.activation(tA[:], tA[:], AF.Ln)
                nc.scalar.activation(tA[:, :, 0], tA[:, :, 0], AF.Exp,
                                     scale=-0.5, bias=float(np.log(c["c2n"])))
                nc.scalar.activation(tA[:, :, 1], tA[:, :, 1], AF.Exp,
                                     scale=-0.5, bias=float(np.log(c["c2p"])))
                tZ = pp.tile([P, N, 2], F32, tag="tZ")
                nc.gpsimd.tensor_tensor(tZ[:, :, 0], I[:], tA[:, :, 0], OP.mult)
                nc.gpsimd.tensor_tensor(tZ[:, :, 1], I[:], tA[:, :, 1], OP.mult)
                tC = pp.tile([P, N, 2], F32, tag="tC")
                nc.scalar.activation(tC[:], tZ[:], AF.Square)
                nc.scalar.activation(tC[:], tC[:], AF.Ln, bias=1.0)
                nc.scalar.activation(tC[:], tC[:], AF.Exp, scale=0.5)
                nc.vector.tensor_tensor(tZ[:], tZ[:], tC[:], OP.add)
                nc.scalar.activation(tZ[:], tZ[:], AF.Ln)

                # overpotentials
                tW = pp.tile([P, N, 2], F32, tag="tW")
                act_affine(tW[:, :, 0], S3[:, :, 0], c["bVsn_cT"])
                act_affine(tW[:, :, 1], S3[:, :, 0], c["bVsp_cT"])
                nc.gpsimd.tensor_tensor(tZ[:], tW[:], tZ[:], OP.mult)
                nc.vector.scalar_tensor_tensor(X3[:, :, 2], S3[:, :, 2], f(c["aVsn"]),
                                               tZ[:, :, 0], OP.mult, OP.add)
                nc.vector.scalar_tensor_tensor(X3[:, :, 3], S3[:, :, 3], f(c["aVsp"]),
                                               tZ[:, :, 1], OP.mult, OP.add)
                tVo = up.tile([P, N], F32, tag="tVo")
                act_affine(tVo[:], S3[:, :, 1], c["aVo"])
                nc.vector.scalar_tensor_tensor(X3[:, :, 1], I[:], f(c["bVo"]),
                                               tVo[:], OP.mult, OP.add)
                nc.vector.tensor_copy(X3[:, :, 0], S3[:, :, 0])   # Tb

                # charge updates
                tG = pp.tile([P, N, 2], F32, tag="tG")
                tH = pp.tile([P, N, 2], F32, tag="tH")
                act_affine(tG[:], S_qB, 1.0 - c4)
                nc.vector.scalar_tensor_tensor(X_qB, S_qS, c5, tG[:], OP.mult, OP.add)
                act_affine(tH[:], S_qS, 1.0 - c5)
                nc.vector.scalar_tensor_tensor(tH[:], S_qB, c4, tH[:], OP.mult, OP.add)
                nc.gpsimd.tensor_tensor(X3[:, :, 5], tH[:, :, 0], I[:], OP.subtract)
                nc.gpsimd.tensor_tensor(X3[:, :, 7], tH[:, :, 1], I[:], OP.add)

                # wh = cT2*Tb (read S now; consumed in stage2)
                wh = mp.tile([P, N], F32, tag="wh")
                act_affine(wh[:], S3[:, :, 0], c["cT2"])
                st_["wh"] = wh

            def stage2(t):
                st_ = state.pop(t)
                X3, wh = st_["X3"], st_["wh"]
                X = st_["X"]
                Vt = xo.tile([P, N], F32, tag="Vt", name="Vt")
                X4 = X[:].rearrange("p (n a b) -> p n a b", a=4, b=2)
                X_qS = X4[:, :, 2:4, 1]

                # t_p and Horner chain (DVE)
                tp = up.tile([P, N], F32, tag="tp")
                nc.vector.tensor_scalar(tp[:], X3[:, :, 7], 2.0 * c1, 1.0,
                                        OP.mult, OP.subtract)
                acc = up.tile([P, N], F32, tag="acc")
                nc.vector.tensor_scalar_mul(acc[:], tp[:], f(etil[13]))
                for k in range(2, 14):
                    nc.vector.scalar_tensor_tensor(acc[:], acc[:], f(etil[14 - k]),
                                                   tp[:], OP.add, OP.mult)

                # logits
                tL1 = pp.tile([P, N, 2], F32, tag="tL1")
                tL2 = pp.tile([P, N, 2], F32, tag="tL2")
                nc.scalar.activation(tL1[:], X_qS, AF.Ln, scale=c1)
                nc.scalar.activation(tL2[:], X_qS, AF.Ln, scale=-c1, bias=1.0)
                nc.vector.tensor_tensor(tL2[:], tL2[:], tL1[:], OP.subtract)
                dd = up.tile([P, N], F32, tag="dd")
                nc.gpsimd.tensor_tensor(dd[:], tL2[:, :, 1], tL2[:, :, 0], OP.subtract)
                nc.gpsimd.tensor_tensor(dd[:], wh[:], dd[:], OP.mult)

                sm = up.tile([P, N], F32, tag="sm")
                nc.gpsimd.tensor_tensor(sm[:], X3[:, :, 1], X3[:, :, 2], OP.add)
                nc.gpsimd.tensor_tensor(sm[:], sm[:], X3[:, :, 3], OP.add)
                act_affine(sm[:], sm[:], -1.0, c["Kc"])
                nc.vector.scalar_tensor_tensor(acc[:], X3[:, :, 5], f(-c["cA"]),
                                               acc[:], OP.mult, OP.add)
                nc.gpsimd.tensor_tensor(acc[:], acc[:], dd[:], OP.add)
                nc.gpsimd.tensor_tensor(Vt[:], acc[:], sm[:], OP.add)

                nc.sync.dma_start(x_v[t], X[:])
                nc.sync.dma_start(v_v[t], Vt[:])

            # software-pipelined emission: loads 1 tick ahead, stage2 skewed
            def whole():
                load(0)
                for k in range(T + SKEW):
                    if k + 1 < T:
                        load(k + 1)
                    if k >= SKEW:
                        stage2(k - SKEW)
                    if k < T:
                        stage1(k)

            if reps == 1:
                whole()
            else:
                with tc.For_i(0, reps, 1):
                    whole()
    nc.compile()
    return nc


def _run(inputs, states, Ap_mult, An0_mult, trace=False):
    c = _derived_consts(Ap_mult, An0_mult)
    nc = build_nc(c)
    inputs = np.ascontiguousarray(np.asarray(inputs, dtype=np.float32))
    states = np.ascontiguousarray(np.asarray(states, dtype=np.float32))
    in_maps = [
        {"inp": inputs[k * SH:(k + 1) * SH], "st": states[k * SH:(k + 1) * SH]}
        for k in range(N_CORES)
    ]
    res = bass_utils.run_bass_kernel_spmd(
        nc, in_maps, core_ids=list(range(N_CORES)), trace=trace
    )
    V = np.concatenate([res.results[k]["V"] for k in range(N_CORES)], axis=0)
    X = np.concatenate([res.results[k]["X"] for k in range(N_CORES)], axis=0)
    return (V, X), res


def kernel(inputs, states, Ap_mult, An0_mult):
    (V, X), _ = _run(inputs, states, Ap_mult, An0_mult, trace=False)
    return V, X


if __name__ == "__main__":
    rng = np.random.default_rng(0)
    inputs = (0.5 + 2.5 * rng.random((B_FULL, 1))).astype(np.float32)
    qMax = QMOBILE / 0.6
    qS, qB = qMax * 0.1, qMax * 0.9
    xn = (0.25 + 0.3 * rng.random(B_FULL)).astype(np.float32)
    xp = (0.45 + 0.3 * rng.random(B_FULL)).astype(np.float32)
    states = np.stack([
        292.1 + 0.5 * rng.random(B_FULL), 0.1 * rng.random(B_FULL),
        0.05 * rng.random(B_FULL), 0.05 * rng.random(B_FULL),
        xn * qB, xn * qS, xp * qB, xp * qS], axis=1).astype(np.float32)
    V, X = kernel(inputs, states, np.ones(13, np.float32), np.ones(1, np.float32))
    print("V", V.shape, V[:4, 0])
    print("X", X.shape, X[0])


# revision 25
# speedup vs baseline: 842.0921x; 1.1451x over previous
"""Trainium2 Bass kernel for the AnalyticalBatteryRNNCell batch step.

Takes FULL inputs (B=2097152 samples), shards the batch across 8 NeuronCores
(pure data parallel), runs a Tile-scheduled elementwise kernel per core, and
gathers the full output: (V[B,1], X_new[B,8]).

Math restructuring vs the jax reference (exact algebra, f32 rounding only):
  - rsqrt(u)        = exp(-0.5*ln(u))          [single ACT table set ln/exp]
  - asinh(z)        = ln(z + exp(0.5*ln(z^2+1)))
  - Redlich-Kister sum for the positive electrode collapses to a single
    degree-13 polynomial R(t) = sum_{j>=1} e_j t^j (constant folded into U0P),
    with e_j = ((j+1)/2)*(Ap[j-1]-Ap[j+1]); evaluated by a fused
    scalar_tensor_tensor Horner chain: acc <- (acc + b_k) * t.
  - negative-electrode RK sum (K=1) is affine in qnS_new and folded into the
    final linear combination.

Schedule: two software-pipelined stages per tile (front / tail), emitted with
a one-tile skew so each engine's in-order queue always has ready work.
Engines: ACT = transcendentals + affine scale/bias; DVE = fused
scalar_tensor_tensor chains (HW-only there); Pool = plain tensor_tensor.
"""
import sys
import numpy as np

for _p in ("/opt/trn_rl_repo", "/root/.axon_site/_ro/trn_rl_repo"):
    if _p not in sys.path:
        sys.path.insert(0, _p)

import concourse.bass as bass
import concourse.bacc as bacc
import concourse.mybir as mybir
import concourse.tile as tile
from concourse import bass_utils

F32 = mybir.dt.float32
AF = mybir.ActivationFunctionType
OP = mybir.AluOpType

# All ACT functions used here (Square/Ln/Exp/Identity/Copy) live in the
# `natural_log_exp_and_others` table set. The stock insert_act_table_loads
# pass picks the FIRST set containing each function, which alternates
# exp_and_others <-> natural_log and reloads the table (~2.7us) on every
# Ln/Exp transition. Restrict the pass to the one covering set (index order
# preserved so act_func_set_id still matches act_info.json).
_ACT_KEEP_SET = "natural_log_exp_and_others"


def _patched_insert_act_table_loads(self):
    import bass_rust as _br
    from concourse.hw_specs import get_activation_tables
    has_activation = any(
        isinstance(i, mybir.InstActivation)
        for b in self.main_func.blocks
        for i in b.instructions
    )
    if not has_activation:
        return
    tables = [
        (nm, (s if nm == _ACT_KEEP_SET else set()))
        for nm, s in get_activation_tables(self.m.arch).items()
    ]
    _br.insert_act_table_loads(self, tables)


bacc.Bacc.insert_act_table_loads = _patched_insert_act_table_loads

B_FULL = 2_097_152
N_CORES = 8
SH = B_FULL // N_CORES          # 262144 samples per core
P = 128
N = 512                         # free elems per partition per tile
T = SH // (P * N)
assert T * P * N == SH

QMOBILE = 7600.0
XN_MAX, XN_MIN = 0.6, 0.0
RO = 0.117215
R_GAS = 8.3144621
F_C = 96487.0
ALPHA = 0.5
SN, SP = 0.000437545, 0.00030962
KN, KP = 2120.96, 248898.0
VOL = 2.0e-5
VOL_S_FRAC = 0.1
T_DIFF = 7.0e6
TO, TSN, TSP = 6.08671, 1001.38, 46.4311
U0P, U0N = 4.03, 0.01
BASE_AP = np.array([-31593.7, 0.106747, 24606.4, -78561.9, 13317.9, 307387.0,
                    84916.1, -1.07469e6, 2285.04, 990894.0, 283920.0,
                    -161513.0, -469218.0], dtype=np.float64)
BASE_AN0 = 86.19


def _derived_consts(Ap_mult, An0_mult):
    qMax = QMOBILE / (XN_MAX - XN_MIN)
    VolS = VOL_S_FRAC * VOL
    VolB = VOL - VolS
    qSMax = qMax * VolS / VOL
    c = {}
    c["c1"] = 1.0 / qSMax
    cT = R_GAS / (F_C * ALPHA)
    c["cT2"] = R_GAS / F_C
    c["c2n"] = 1.0 / (SN * 2.0 * KN)
    c["c2p"] = 1.0 / (SP * 2.0 * KP)
    c["c4"] = 1.0 / (VolB * T_DIFF)
    c["c5"] = 1.0 / (VolS * T_DIFF)
    c["aVo"], c["bVo"] = 1.0 - 1.0 / TO, RO / TO
    c["aVsn"], c["bVsn_cT"] = 1.0 - 1.0 / TSN, cT / TSN
    c["aVsp"], c["bVsp_cT"] = 1.0 - 1.0 / TSP, cT / TSP

    Ap = np.float64(np.asarray(Ap_mult, dtype=np.float64)) * BASE_AP
    An0 = float(np.asarray(An0_mult).reshape(-1)[0]) * BASE_AN0
    A = np.zeros(15)
    A[:13] = Ap
    e = np.zeros(14)
    for j in range(14):
        am1 = A[j - 1] if j >= 1 else 0.0
        e[j] = 0.5 * (j + 1) * (am1 - A[j + 1])
    c["etil"] = e / F_C
    c["cA"] = 2.0 * An0 * c["c1"] / F_C
    c["Kc"] = (U0P - U0N) + e[0] / F_C + An0 / F_C
    return c


def build_nc(c, reps=1):
    nc = bacc.Bacc(
        "TRN2", target_bir_lowering=False, debug=False, num_devices=N_CORES
    )

    f = lambda x: float(x)
    c1, c4, c5 = f(c["c1"]), f(c["c4"]), f(c["c5"])
    etil = c["etil"]

    act_biases = [float(np.log(c["c2n"])), float(np.log(c["c2p"])), f(c["Kc"]), -1.0]
    for val in act_biases:
        if val in (0.0, 1.0):
            continue
        tns = nc.alloc_sbuf_tensor(f"constb-{val!r}", [128, 1], F32)
        nc.gpsimd.memset(tns.ap(), val)
        nc.const_aps.aps[(F32, val)] = tns.ap()
    nc.all_engine_barrier()

    inp_d = nc.dram_tensor("inp", [SH, 1], F32, kind="ExternalInput")
    st_d = nc.dram_tensor("st", [SH, 8], F32, kind="ExternalInput")
    v_d = nc.dram_tensor("V", [SH, 1], F32, kind="ExternalOutput")
    x_d = nc.dram_tensor("X", [SH, 8], F32, kind="ExternalOutput")

    st_v = st_d[:].rearrange("(t p n) c -> t p (n c)", p=P, n=N)
    x_v = x_d[:].rearrange("(t p n) c -> t p (n c)", p=P, n=N)
    inp_v = inp_d[:].rearrange("(t p n) c -> t p (n c)", p=P, n=N)
    v_v = v_d[:].rearrange("(t p n) c -> t p (n c)", p=P, n=N)

    def act_affine(out, in_, s0, s1=0.0):
        if s1 == 0.0:
            nc.scalar.activation(out, in_, AF.Copy, bias=0.0, scale=f(s0))
        else:
            nc.scalar.activation(out, in_, AF.Identity, bias=f(s1), scale=f(s0))

    SKEW = 2     # stage2(t) emitted SKEW ticks after stage1(t)
    with tile.TileContext(nc) as tc:
        with (
            tc.tile_pool(name="sin", bufs=2) as sin,
            tc.tile_pool(name="xo", bufs=SKEW + 2) as xo,
            tc.tile_pool(name="pairs", bufs=2) as pp,
            tc.tile_pool(name="mids", bufs=SKEW + 2) as mp,
            tc.tile_pool(name="units", bufs=2) as up,
        ):
            state = {}       # per-tile live tiles passed stage1 -> stage2

            def load(t):
                S = sin.tile([P, N * 8], F32, tag="S")
                I = sin.tile([P, N], F32, tag="I")
                nc.sync.dma_start(S[:], st_v[t])
                nc.sync.dma_start(I[:], inp_v[t])
                state[t] = {"S": S, "I": I}

            def stage1(t):
                st_ = state[t]
                S, I = st_["S"], st_["I"]
                X = xo.tile([P, N * 8], F32, tag="X")
                st_["X"] = X
                S3 = S[:].rearrange("p (n c) -> p n c", c=8)
                S4 = S[:].rearrange("p (n a b) -> p n a b", a=4, b=2)
                X3 = X[:].rearrange("p (n c) -> p n c", c=8)
                X4 = X[:].rearrange("p (n a b) -> p n a b", a=4, b=2)
                S_qS, S_qB = S4[:, :, 2:4, 1], S4[:, :, 2:4, 0]
                X_qS, X_qB = X4[:, :, 2:4, 1], X4[:, :, 2:4, 0]
                st_["X3"] = X3

                # transcendental front
                tA = pp.tile([P, N, 2], F32, tag="tA")
                nc.scalar.activation(tA[:], S_qS, AF.Square, scale=c1)
                nc.vector.scalar_tensor_tensor(tA[:], S_qS, c1, tA[:],
                                               OP.mult, OP.subtract)
                nc.scalar.activation(tA[:], tA[:], AF.Ln)
                nc.scalar.activation(tA[:, :, 0], tA[:, :, 0], AF.Exp,
                                     scale=-0.5, bias=float(np.log(c["c2n"])))
                nc.scalar.activation(tA[:, :, 1], tA[:, :, 1], AF.Exp,
                                     scale=-0.5, bias=float(np.log(c["c2p"])))
                tZ = pp.tile([P, N, 2], F32, tag="tZ")
                nc.vector.tensor_tensor(tZ[:, :, 0], I[:], tA[:, :, 0], OP.mult)
                nc.vector.tensor_tensor(tZ[:, :, 1], I[:], tA[:, :, 1], OP.mult)
                tC = pp.tile([P, N, 2], F32, tag="tC")
                nc.scalar.activation(tC[:], tZ[:], AF.Square)
                nc.scalar.activation(tC[:], tC[:], AF.Ln, bias=1.0)
                nc.scalar.activation(tC[:], tC[:], AF.Exp, scale=0.5)
                nc.vector.tensor_tensor(tZ[:], tZ[:], tC[:], OP.add)
                nc.scalar.activation(tZ[:], tZ[:], AF.Ln)

                # overpotentials
                tW = pp.tile([P, N, 2], F32, tag="tW")
                act_affine(tW[:, :, 0], S3[:, :, 0], c["bVsn_cT"])
                act_affine(tW[:, :, 1], S3[:, :, 0], c["bVsp_cT"])
                nc.vector.tensor_tensor(tZ[:], tW[:], tZ[:], OP.mult)
                nc.vector.scalar_tensor_tensor(X3[:, :, 2], S3[:, :, 2], f(c["aVsn"]),
                                               tZ[:, :, 0], OP.mult, OP.add)
                nc.vector.scalar_tensor_tensor(X3[:, :, 3], S3[:, :, 3], f(c["aVsp"]),
                                               tZ[:, :, 1], OP.mult, OP.add)
                tVo = up.tile([P, N], F32, tag="tVo")
                act_affine(tVo[:], S3[:, :, 1], c["aVo"])
                nc.vector.scalar_tensor_tensor(X3[:, :, 1], I[:], f(c["bVo"]),
                                               tVo[:], OP.mult, OP.add)
                nc.scalar.copy(X3[:, :, 0], S3[:, :, 0])   # Tb

                # charge updates
                tG = pp.tile([P, N, 2], F32, tag="tG")
                tH = pp.tile([P, N, 2], F32, tag="tH")
                act_affine(tG[:], S_qB, 1.0 - c4)
                nc.vector.scalar_tensor_tensor(X_qB, S_qS, c5, tG[:], OP.mult, OP.add)
                act_affine(tH[:], S_qS, 1.0 - c5)
                nc.vector.scalar_tensor_tensor(tH[:], S_qB, c4, tH[:], OP.mult, OP.add)
                nc.vector.tensor_tensor(X3[:, :, 5], tH[:, :, 0], I[:], OP.subtract)
                nc.vector.tensor_tensor(X3[:, :, 7], tH[:, :, 1], I[:], OP.add)

                # wh = cT2*Tb (read S now; consumed in stage2)
                wh = mp.tile([P, N], F32, tag="wh")
                act_affine(wh[:], S3[:, :, 0], c["cT2"])
                st_["wh"] = wh

            def stage2(t):
                st_ = state.pop(t)
                X3, wh = st_["X3"], st_["wh"]
                X = st_["X"]
                X4 = X[:].rearrange("p (n a b) -> p n a b", a=4, b=2)
                qn = X4[:, :, 2:4, 1]
                Vt = xo.tile([P, N], F32, tag="Vt", name="Vt")

                # t_p and Horner chain (DVE)
                tp = up.tile([P, N], F32, tag="tp")
                act_affine(tp[:], X3[:, :, 7], 2.0 * c1, -1.0)
                acc = up.tile([P, N], F32, tag="acc")
                act_affine(acc[:], tp[:], f(etil[13]))
                for k in range(2, 14):
                    nc.vector.scalar_tensor_tensor(acc[:], acc[:], f(etil[14 - k]),
                                                   tp[:], OP.add, OP.mult)

                # logits
                tL1 = pp.tile([P, N, 2], F32, tag="tA")
                tL2 = pp.tile([P, N, 2], F32, tag="tC")
                nc.scalar.activation(tL1[:], qn, AF.Ln, scale=c1)
                nc.scalar.activation(tL2[:], qn, AF.Ln, scale=-c1, bias=1.0)
                nc.vector.tensor_tensor(tL2[:], tL2[:], tL1[:], OP.subtract)
                dd = up.tile([P, N], F32, tag="dd")
                nc.vector.tensor_tensor(dd[:], tL2[:, :, 1], tL2[:, :, 0], OP.subtract)
                nc.vector.tensor_tensor(dd[:], wh[:], dd[:], OP.mult)

                sm = up.tile([P, N], F32, tag="sm")
                nc.vector.tensor_tensor(sm[:], X3[:, :, 1], X3[:, :, 2], OP.add)
                nc.vector.tensor_tensor(sm[:], sm[:], X3[:, :, 3], OP.add)
                act_affine(sm[:], sm[:], -1.0, c["Kc"])
                nc.vector.scalar_tensor_tensor(acc[:], X3[:, :, 5], f(-c["cA"]),
                                               acc[:], OP.mult, OP.add)
                nc.vector.tensor_tensor(acc[:], acc[:], dd[:], OP.add)
                nc.vector.tensor_tensor(Vt[:], acc[:], sm[:], OP.add)

                nc.sync.dma_start(x_v[t], X[:])
                nc.sync.dma_start(v_v[t], Vt[:])

            # software-pipelined emission: loads 1 tick ahead, stage2 skewed
            def whole():
                load(0)
                for k in range(T + SKEW):
                    if k + 1 < T:
                        load(k + 1)
                    if k >= SKEW:
                        stage2(k - SKEW)
                    if k < T:
                        stage1(k)

            if reps == 1:
                whole()
            else:
                with tc.For_i(0, reps, 1):
                    whole()
    nc.compile()
    return nc


def _run(inputs, states, Ap_mult, An0_mult, trace=False):
    c = _derived_consts(Ap_mult, An0_mult)
    nc = build_nc(c)
    inputs = np.ascontiguousarray(np.asarray(inputs, dtype=np.float32))
    states = np.ascontiguousarray(np.asarray(states, dtype=np.float32))
    in_maps = [
        {"inp": inputs[k * SH:(k + 1) * SH], "st": states[k * SH:(k + 1) * SH]}
        for k in range(N_CORES)
    ]
    res = bass_utils.run_bass_kernel_spmd(
        nc, in_maps, core_ids=list(range(N_CORES)), trace=trace
    )
    V = np.concatenate([res.results[k]["V"] for k in range(N_CORES)], axis=0)
    X = np.concatenate([res.results[k]["X"] for k in range(N_CORES)], axis=0)
    return (V, X), res


def kernel(inputs, states, Ap_mult, An0_mult):
    (V, X), _ = _run(inputs, states, Ap_mult, An0_mult, trace=False)
    return V, X


if __name__ == "__main__":
    rng = np.random.default_rng(0)
    inputs = (0.5 + 2.5 * rng.random((B_FULL, 1))).astype(np.float32)
    qMax = QMOBILE / 0.6
    qS, qB = qMax * 0.1, qMax * 0.9
    xn = (0.25 + 0.3 * rng.random(B_FULL)).astype(np.float32)
    xp = (0.45 + 0.3 * rng.random(B_FULL)).astype(np.float32)
    states = np.stack([
        292.1 + 0.5 * rng.random(B_FULL), 0.1 * rng.random(B_FULL),
        0.05 * rng.random(B_FULL), 0.05 * rng.random(B_FULL),
        xn * qB, xn * qS, xp * qB, xp * qS], axis=1).astype(np.float32)
    V, X = kernel(inputs, states, np.ones(13, np.float32), np.ones(1, np.float32))
    print("V", V.shape, V[:4, 0])
    print("X", X.shape, X[0])


# revision 31
# speedup vs baseline: 1082.8417x; 1.2859x over previous
"""Trainium2 Bass kernel for the AnalyticalBatteryRNNCell batch step.

Takes FULL inputs (B=2097152 samples), shards the batch across 8 NeuronCores
(pure data parallel), runs a Tile-scheduled elementwise kernel per core, and
gathers the full output: (V[B,1], X_new[B,8]).

Math restructuring vs the jax reference (exact algebra, f32 rounding only):
  - rsqrt(u)        = exp(-0.5*ln(u))          [single ACT table set ln/exp]
  - asinh(z)        = ln(z + exp(0.5*ln(z^2+1)))
  - Redlich-Kister sum for the positive electrode collapses to a single
    degree-13 polynomial R(t) = sum_{j>=1} e_j t^j (constant folded into U0P),
    with e_j = ((j+1)/2)*(Ap[j-1]-Ap[j+1]); evaluated by a fused
    scalar_tensor_tensor Horner chain: acc <- (acc + b_k) * t.
  - negative-electrode RK sum (K=1) is affine in qnS_new and folded into the
    final linear combination.

Schedule: two software-pipelined stages per tile (front / tail), emitted with
a one-tile skew so each engine's in-order queue always has ready work.
Engines: ACT = transcendentals + affine scale/bias; DVE = fused
scalar_tensor_tensor chains (HW-only there); Pool = plain tensor_tensor.
"""
import sys
import numpy as np

for _p in ("/opt/trn_rl_repo", "/root/.axon_site/_ro/trn_rl_repo"):
    if _p not in sys.path:
        sys.path.insert(0, _p)

import concourse.bass as bass
import concourse.bacc as bacc
import concourse.mybir as mybir
import concourse.tile as tile
from concourse import bass_utils

F32 = mybir.dt.float32
AF = mybir.ActivationFunctionType
OP = mybir.AluOpType

# All ACT functions used here (Square/Ln/Exp/Identity/Copy) live in the
# `natural_log_exp_and_others` table set. The stock insert_act_table_loads
# pass picks the FIRST set containing each function, which alternates
# exp_and_others <-> natural_log and reloads the table (~2.7us) on every
# Ln/Exp transition. Restrict the pass to the one covering set (index order
# preserved so act_func_set_id still matches act_info.json).
_ACT_KEEP_SET = "natural_log_exp_and_others"


def _patched_insert_act_table_loads(self):
    import bass_rust as _br
    from concourse.hw_specs import get_activation_tables
    has_activation = any(
        isinstance(i, mybir.InstActivation)
        for b in self.main_func.blocks
        for i in b.instructions
    )
    if not has_activation:
        return
    tables = [
        (nm, (s if nm == _ACT_KEEP_SET else set()))
        for nm, s in get_activation_tables(self.m.arch).items()
    ]
    _br.insert_act_table_loads(self, tables)


bacc.Bacc.insert_act_table_loads = _patched_insert_act_table_loads

B_FULL = 2_097_152
N_CORES = 8
SH = B_FULL // N_CORES          # 262144 samples per core
P = 128
N = 512                         # free elems per partition per tile
T = SH // (P * N)
assert T * P * N == SH

QMOBILE = 7600.0
XN_MAX, XN_MIN = 0.6, 0.0
RO = 0.117215
R_GAS = 8.3144621
F_C = 96487.0
ALPHA = 0.5
SN, SP = 0.000437545, 0.00030962
KN, KP = 2120.96, 248898.0
VOL = 2.0e-5
VOL_S_FRAC = 0.1
T_DIFF = 7.0e6
TO, TSN, TSP = 6.08671, 1001.38, 46.4311
U0P, U0N = 4.03, 0.01
BASE_AP = np.array([-31593.7, 0.106747, 24606.4, -78561.9, 13317.9, 307387.0,
                    84916.1, -1.07469e6, 2285.04, 990894.0, 283920.0,
                    -161513.0, -469218.0], dtype=np.float64)
BASE_AN0 = 86.19


def _derived_consts(Ap_mult, An0_mult):
    qMax = QMOBILE / (XN_MAX - XN_MIN)
    VolS = VOL_S_FRAC * VOL
    VolB = VOL - VolS
    qSMax = qMax * VolS / VOL
    c = {}
    c["c1"] = 1.0 / qSMax
    cT = R_GAS / (F_C * ALPHA)
    c["cT2"] = R_GAS / F_C
    c["c2n"] = 1.0 / (SN * 2.0 * KN)
    c["c2p"] = 1.0 / (SP * 2.0 * KP)
    c["c4"] = 1.0 / (VolB * T_DIFF)
    c["c5"] = 1.0 / (VolS * T_DIFF)
    c["aVo"], c["bVo"] = 1.0 - 1.0 / TO, RO / TO
    c["aVsn"], c["bVsn_cT"] = 1.0 - 1.0 / TSN, cT / TSN
    c["aVsp"], c["bVsp_cT"] = 1.0 - 1.0 / TSP, cT / TSP

    Ap = np.float64(np.asarray(Ap_mult, dtype=np.float64)) * BASE_AP
    An0 = float(np.asarray(An0_mult).reshape(-1)[0]) * BASE_AN0
    A = np.zeros(15)
    A[:13] = Ap
    e = np.zeros(14)
    for j in range(14):
        am1 = A[j - 1] if j >= 1 else 0.0
        e[j] = 0.5 * (j + 1) * (am1 - A[j + 1])
    c["etil"] = e / F_C
    c["cA"] = 2.0 * An0 * c["c1"] / F_C
    c["Kc"] = (U0P - U0N) + e[0] / F_C + An0 / F_C
    return c


def build_nc(c, reps=1):
    nc = bacc.Bacc(
        "TRN2", target_bir_lowering=False, debug=False, num_devices=N_CORES
    )

    f = lambda x: float(x)
    c1, c4, c5 = f(c["c1"]), f(c["c4"]), f(c["c5"])
    etil = c["etil"]

    act_biases = [float(np.log(c["c2n"])), float(np.log(c["c2p"])), f(c["Kc"]), -1.0]
    for val in act_biases:
        if val in (0.0, 1.0):
            continue
        tns = nc.alloc_sbuf_tensor(f"constb-{val!r}", [128, 1], F32)
        nc.gpsimd.memset(tns.ap(), val)
        nc.const_aps.aps[(F32, val)] = tns.ap()
    nc.all_engine_barrier()

    inp_d = nc.dram_tensor("inp", [SH, 1], F32, kind="ExternalInput")
    st_d = nc.dram_tensor("st", [SH, 8], F32, kind="ExternalInput")
    v_d = nc.dram_tensor("V", [SH, 1], F32, kind="ExternalOutput")
    x_d = nc.dram_tensor("X", [SH, 8], F32, kind="ExternalOutput")

    st_v = st_d[:].rearrange("(t p n) c -> t p (n c)", p=P, n=N)
    x_v = x_d[:].rearrange("(t p n) c -> t p (n c)", p=P, n=N)
    inp_v = inp_d[:].rearrange("(t p n) c -> t p (n c)", p=P, n=N)
    v_v = v_d[:].rearrange("(t p n) c -> t p (n c)", p=P, n=N)

    def act_affine(out, in_, s0, s1=0.0):
        if s1 == 0.0:
            nc.scalar.activation(out, in_, AF.Copy, bias=0.0, scale=f(s0))
        else:
            nc.scalar.activation(out, in_, AF.Identity, bias=f(s1), scale=f(s0))

    SKEW = 2     # stage2(t) emitted SKEW ticks after stage1(t)
    with tile.TileContext(nc) as tc:
        with (
            tc.tile_pool(name="sin", bufs=2) as sin,
            tc.tile_pool(name="xo", bufs=SKEW + 2) as xo,
            tc.tile_pool(name="pairs", bufs=2) as pp,
            tc.tile_pool(name="mids", bufs=SKEW + 2) as mp,
            tc.tile_pool(name="units", bufs=2) as up,
            tc.tile_pool(name="psum", bufs=2, space="PSUM") as ps,
        ):
            state = {}       # per-tile live tiles passed stage1 -> stage2

            def load(t):
                S = sin.tile([P, N * 8], F32, tag="S")
                I = sin.tile([P, N], F32, tag="I")
                nc.sync.dma_start(S[:], st_v[t])
                nc.sync.dma_start(I[:], inp_v[t])
                state[t] = {"S": S, "I": I}

            def stage1(t):
                st_ = state[t]
                S, I = st_["S"], st_["I"]
                X = xo.tile([P, N * 8], F32, tag="X")
                st_["X"] = X
                S3 = S[:].rearrange("p (n c) -> p n c", c=8)
                S4 = S[:].rearrange("p (n a b) -> p n a b", a=4, b=2)
                X3 = X[:].rearrange("p (n c) -> p n c", c=8)
                X4 = X[:].rearrange("p (n a b) -> p n a b", a=4, b=2)
                S_qS, S_qB = S4[:, :, 2:4, 1], S4[:, :, 2:4, 0]
                X_qS, X_qB = X4[:, :, 2:4, 1], X4[:, :, 2:4, 0]
                st_["X3"] = X3

                # transcendental front
                tA = pp.tile([P, N, 2], F32, tag="tA")
                nc.scalar.activation(tA[:], S_qS, AF.Square, scale=c1)
                nc.vector.scalar_tensor_tensor(tA[:], S_qS, c1, tA[:],
                                               OP.mult, OP.subtract)
                nc.scalar.activation(tA[:], tA[:], AF.Ln)
                nc.scalar.activation(tA[:, :, 0], tA[:, :, 0], AF.Exp,
                                     scale=-0.5, bias=float(np.log(c["c2n"])))
                nc.scalar.activation(tA[:, :, 1], tA[:, :, 1], AF.Exp,
                                     scale=-0.5, bias=float(np.log(c["c2p"])))
                tZ = pp.tile([P, N, 2], F32, tag="tZ")
                nc.vector.tensor_tensor(tZ[:, :, 0], I[:], tA[:, :, 0], OP.mult)
                nc.vector.tensor_tensor(tZ[:, :, 1], I[:], tA[:, :, 1], OP.mult)
                tC = pp.tile([P, N, 2], F32, tag="tC")
                nc.scalar.activation(tC[:], tZ[:], AF.Square)
                nc.scalar.activation(tC[:], tC[:], AF.Ln, bias=1.0)
                nc.scalar.activation(tC[:], tC[:], AF.Exp, scale=0.5)
                nc.gpsimd.tensor_tensor(tZ[:], tZ[:], tC[:], OP.add)
                nc.scalar.activation(tZ[:], tZ[:], AF.Ln)

                # overpotentials
                tW = pp.tile([P, N, 2], F32, tag="tW")
                act_affine(tW[:, :, 0], S3[:, :, 0], c["bVsn_cT"])
                act_affine(tW[:, :, 1], S3[:, :, 0], c["bVsp_cT"])
                nc.gpsimd.tensor_tensor(tZ[:], tW[:], tZ[:], OP.mult)
                nc.vector.scalar_tensor_tensor(X3[:, :, 2], S3[:, :, 2], f(c["aVsn"]),
                                               tZ[:, :, 0], OP.mult, OP.add)
                nc.vector.scalar_tensor_tensor(X3[:, :, 3], S3[:, :, 3], f(c["aVsp"]),
                                               tZ[:, :, 1], OP.mult, OP.add)
                tVo = up.tile([P, N], F32, tag="tVo")
                act_affine(tVo[:], S3[:, :, 1], c["aVo"])
                nc.vector.scalar_tensor_tensor(X3[:, :, 1], I[:], f(c["bVo"]),
                                               tVo[:], OP.mult, OP.add)
                nc.scalar.copy(X3[:, :, 0], S3[:, :, 0])   # Tb

                # charge updates
                tG = pp.tile([P, N, 2], F32, tag="tG")
                tH = pp.tile([P, N, 2], F32, tag="tH")
                act_affine(tG[:], S_qB, 1.0 - c4)
                nc.vector.scalar_tensor_tensor(X_qB, S_qS, c5, tG[:], OP.mult, OP.add)
                act_affine(tH[:], S_qS, 1.0 - c5)
                nc.vector.scalar_tensor_tensor(tH[:], S_qB, c4, tH[:], OP.mult, OP.add)
                nc.gpsimd.tensor_tensor(X3[:, :, 5], tH[:, :, 0], I[:], OP.subtract)
                nc.gpsimd.tensor_tensor(X3[:, :, 7], tH[:, :, 1], I[:], OP.add)

                # wh = cT2*Tb (read S now; consumed in stage2)
                wh = mp.tile([P, N], F32, tag="wh")
                act_affine(wh[:], S3[:, :, 0], c["cT2"])
                st_["wh"] = wh

            def stage2(t):
                st_ = state.pop(t)
                X3, wh = st_["X3"], st_["wh"]
                X = st_["X"]
                X4 = X[:].rearrange("p (n a b) -> p n a b", a=4, b=2)
                qn = X4[:, :, 2:4, 1]
                Vt = xo.tile([P, N], F32, tag="Vt", name="Vt")

                # t_p and Horner chain (DVE)
                tp = up.tile([P, N], F32, tag="tp")
                act_affine(tp[:], X3[:, :, 7], 2.0 * c1, -1.0)
                acc = ps.tile([P, N], F32, tag="acc")
                act_affine(acc[:], tp[:], f(etil[13]))
                for k in range(2, 14):
                    nc.vector.scalar_tensor_tensor(acc[:], acc[:], f(etil[14 - k]),
                                                   tp[:], OP.add, OP.mult)

                # logits
                tL1 = pp.tile([P, N, 2], F32, tag="tA")
                tL2 = pp.tile([P, N, 2], F32, tag="tC")
                nc.scalar.activation(tL1[:], qn, AF.Ln, scale=c1)
                nc.scalar.activation(tL2[:], qn, AF.Ln, scale=-c1, bias=1.0)
                nc.gpsimd.tensor_tensor(tL2[:], tL2[:], tL1[:], OP.subtract)
                dd = up.tile([P, N], F32, tag="dd")
                nc.gpsimd.tensor_tensor(dd[:], tL2[:, :, 1], tL2[:, :, 0], OP.subtract)
                nc.gpsimd.tensor_tensor(dd[:], wh[:], dd[:], OP.mult)

                sm = up.tile([P, N], F32, tag="sm")
                nc.gpsimd.tensor_tensor(sm[:], X3[:, :, 1], X3[:, :, 2], OP.add)
                nc.gpsimd.tensor_tensor(sm[:], sm[:], X3[:, :, 3], OP.add)
                act_affine(sm[:], sm[:], -1.0, c["Kc"])
                nc.vector.scalar_tensor_tensor(acc[:], X3[:, :, 5], f(-c["cA"]),
                                               acc[:], OP.mult, OP.add)
                nc.vector.tensor_tensor(acc[:], acc[:], dd[:], OP.add)
                nc.vector.tensor_tensor(Vt[:], acc[:], sm[:], OP.add)

                nc.sync.dma_start(x_v[t], X[:])
                nc.sync.dma_start(v_v[t], Vt[:])

            # software-pipelined emission: loads 1 tick ahead, stage2 skewed
            def whole():
                load(0)
                for k in range(T + SKEW):
                    if k + 1 < T:
                        load(k + 1)
                    if k >= SKEW:
                        stage2(k - SKEW)
                    if k < T:
                        stage1(k)

            if reps == 1:
                whole()
            else:
                with tc.For_i(0, reps, 1):
                    whole()
    nc.compile()
    return nc


def _run(inputs, states, Ap_mult, An0_mult, trace=False):
    c = _derived_consts(Ap_mult, An0_mult)
    nc = build_nc(c)
    inputs = np.ascontiguousarray(np.asarray(inputs, dtype=np.float32))
    states = np.ascontiguousarray(np.asarray(states, dtype=np.float32))
    in_maps = [
        {"inp": inputs[k * SH:(k + 1) * SH], "st": states[k * SH:(k + 1) * SH]}
        for k in range(N_CORES)
    ]
    res = bass_utils.run_bass_kernel_spmd(
        nc, in_maps, core_ids=list(range(N_CORES)), trace=trace
    )
    V = np.concatenate([res.results[k]["V"] for k in range(N_CORES)], axis=0)
    X = np.concatenate([res.results[k]["X"] for k in range(N_CORES)], axis=0)
    return (V, X), res


def kernel(inputs, states, Ap_mult, An0_mult):
    (V, X), _ = _run(inputs, states, Ap_mult, An0_mult, trace=False)
    return V, X


if __name__ == "__main__":
    rng = np.random.default_rng(0)
    inputs = (0.5 + 2.5 * rng.random((B_FULL, 1))).astype(np.float32)
    qMax = QMOBILE / 0.6
    qS, qB = qMax * 0.1, qMax * 0.9
    xn = (0.25 + 0.3 * rng.random(B_FULL)).astype(np.float32)
    xp = (0.45 + 0.3 * rng.random(B_FULL)).astype(np.float32)
    states = np.stack([
        292.1 + 0.5 * rng.random(B_FULL), 0.1 * rng.random(B_FULL),
        0.05 * rng.random(B_FULL), 0.05 * rng.random(B_FULL),
        xn * qB, xn * qS, xp * qB, xp * qS], axis=1).astype(np.float32)
    V, X = kernel(inputs, states, np.ones(13, np.float32), np.ones(1, np.float32))
    print("V", V.shape, V[:4, 0])
    print("X", X.shape, X[0])
